# revision 14
# baseline (speedup 1.0000x reference)
"""Trainium2 Bass kernel for nn_BatchDifferentiableKF.

Problem: batched 4-state Kalman filter, B=16384 batch rows, T=512 steps,
state [px, py, vx, vy], measurements = predicted velocities (B, T, 2).

Key structure exploited:
  * The covariance/gain recursion is data-independent (P0 = I for every
    batch row), so the Kalman gains k_p[t], k_v[t] are a fixed schedule
    computed on host in float64.
  * x/y components decouple into two identical scalar filters:
        p_t = p_{t-1} + g[t] v_{t-1} + k_p[t] z_t      (g = dt - k_p)
        v_t = a[t] v_{t-1} + k_v[t] z_t                (a = 1 - k_v)
    i.e. the whole filter is LINEAR in (z, p0).
  * Chunk T into 4 x 128. Within a chunk the map z -> outputs is a dense
    lower-triangular 128x128 matrix pair (Wp, Wv); chunks 1..3 share
    identical (steady-state) weights. The cross-chunk carry enters as
        p_out[i] += p_in + Bv[i] v_in ;  v_out[i] += Av[i] v_in
    which is applied in BATCH-MAJOR form during PSUM evacuation: the
    carry scalars (p_in, v_in) are the t=127 columns of the previous
    chunk's staged output, used as per-partition scale/bias operands of
    ALU/ACT ops against constant Bv/Av row-broadcast tiles. No
    transposed carry state, no carry matmuls.

Device algorithm per core (2048 batch rows, 16 batch-tiles):
  1. DMA z fp32 tile, DVE-cast to bf16, PE-transpose (8x 128x128) into
     time-major tiles ztm[m] [128 (s,c) x 2, b], batched 4-per-PSUM.
  2. Per (tile j, chunk m): 2 PE matmuls (bf16 data stationary, weights
     moving) produce out[128 b, 512 (t,c,pv)] in PSUM; DVE adds the
     carry planes (built on GpSimd/Scalar from the previous chunk's
     staged columns) while evacuating to SBUF stage tiles.
  3. Per-tile output DMAs: pos on the scalar HWDGE ring, vel on the
     gpsimd SWDGE ring; z loads on the sync HWDGE ring.

Sharding: embarrassingly parallel over batch across the 8 cores.
"""

import numpy as np
import ml_dtypes

B_FULL = 16384
T = 512
C = 128          # chunk length
NCH = T // C     # 4 chunks
N_CORES = 8
B_CORE = B_FULL // N_CORES   # 2048


# ----------------------------------------------------------------------------
# Host-side weight construction (float64)
# ----------------------------------------------------------------------------

def _gains(dt, q_pos, q_vel, r_vel, n):
    """Gain schedule k_p[t], k_v[t] of the decoupled scalar filter, P0=I."""
    dt = float(np.float32(dt))
    r = float(np.float32(r_vel)) + float(np.float32(1e-6))
    qp = float(np.float32(q_pos))
    qv = float(np.float32(q_vel))
    Ppp, Ppv, Pvv = 1.0, 0.0, 1.0
    k_p = np.zeros(n)
    k_v = np.zeros(n)
    for t in range(n):
        Ppv_ = Ppv + dt * Pvv
        Ppp_ = Ppp + 2.0 * dt * Ppv + dt * dt * Pvv + qp
        Pvv_ = Pvv + qv
        S = Pvv_ + r
        k_p[t] = Ppv_ / S
        k_v[t] = Pvv_ / S
        Ppp = Ppp_ - k_p[t] * Ppv_
        Ppv = Ppv_ - k_p[t] * Pvv_
        Pvv = Pvv_ - k_v[t] * Pvv_
    return k_p, k_v


def _chunk_maps(k_p, k_v, dt):
    """Per-chunk affine maps: (p_in, v_in, z[0..C-1]) -> (p[0..C-1], v[..]).

    p_out[i] = p_in + Bv[m][i] v_in + sum_j Wp[m][i,j] z[j]
    v_out[i] =        Av[m][i] v_in + sum_j Wv[m][i,j] z[j]
    """
    g = dt - k_p
    a = 1.0 - k_v
    Wp = np.zeros((NCH, C, C))
    Wv = np.zeros((NCH, C, C))
    Av = np.zeros((NCH, C))
    Bv = np.zeros((NCH, C))
    for m in range(NCH):
        pcoef = np.zeros(C + 1)
        vcoef = np.zeros(C + 1)
        vcoef[0] = 1.0
        for i in range(C):
            t = m * C + i
            pcoef = pcoef + g[t] * vcoef
            pcoef[1 + i] += k_p[t]
            vcoef = a[t] * vcoef
            vcoef[1 + i] += k_v[t]
            Bv[m, i] = pcoef[0]
            Wp[m, i] = pcoef[1:]
            Av[m, i] = vcoef[0]
            Wv[m, i] = vcoef[1:]
    return Wp, Wv, Av, Bv


def build_weights(dt, q_pos, q_vel, r_vel):
    """Device constant tensors. Layouts:

    partition index q of a transposed-data tile <-> (j_local = 64h + q//2,
    c = q&1) for tile half h; output free index f = pv*256 + t*2 + c'.
    bvrow/avrow: [128, 128] f32 with identical rows Bv[1]/Av[1] (steady
    chunk map), consumed with per-partition scale/bias carries.
    """
    dtf = float(np.float32(dt))
    k_p, k_v = _gains(dt, q_pos, q_vel, r_vel, T)
    Wp, Wv, Av, Bv = _chunk_maps(k_p, k_v, dtf)

    bf16 = ml_dtypes.bfloat16
    out = {}
    for mset in range(2):
        mc = mset  # chunk-map index (chunk 0, or steady chunk 1)
        for h in range(2):
            w = np.zeros((128, 512))
            for q in range(128):
                j = 64 * h + q // 2
                c = q & 1
                # f = pv*256 + t*2 + c ; delta_{c,c'} keeps only c'==c
                w[q, 0 * 256 + 2 * np.arange(C) + c] = Wp[mc, :, j]
                w[q, 1 * 256 + 2 * np.arange(C) + c] = Wv[mc, :, j]
            out[f"wmain_{mset}_{h}"] = w.astype(bf16)
    out["bvrow"] = np.broadcast_to(Bv[1], (128, C)).astype(np.float32).copy()
    out["avrow"] = np.broadcast_to(Av[1], (128, C)).astype(np.float32).copy()
    out["identb"] = np.eye(128, dtype=bf16)
    return out


# ----------------------------------------------------------------------------
# Bass kernel
# ----------------------------------------------------------------------------

def build_nc(n_bt):
    """Build the Bass program for one core processing n_bt*128 batch rows."""
    import concourse.bass as bass
    import concourse.tile as tile
    from concourse import bacc, mybir
    from contextlib import ExitStack

    f32 = mybir.dt.float32
    bf16 = mybir.dt.bfloat16
    ADD = mybir.AluOpType.add
    MULT = mybir.AluOpType.mult
    IDENT = mybir.ActivationFunctionType.Identity

    b_sz = n_bt * 128
    nc = bacc.Bacc("TRN2", target_bir_lowering=False, debug=False)

    z_in = nc.dram_tensor("z_in", [b_sz, 1024], f32, kind="ExternalInput").ap()
    p0_in = nc.dram_tensor("p0_in", [b_sz, 2], f32, kind="ExternalInput").ap()
    wmain_d = [[nc.dram_tensor(f"wmain_{ms}_{h}", [128, 512], bf16,
                               kind="ExternalInput").ap()
                for h in range(2)] for ms in range(2)]
    bvrow_d = nc.dram_tensor("bvrow", [128, 128], f32, kind="ExternalInput").ap()
    avrow_d = nc.dram_tensor("avrow", [128, 128], f32, kind="ExternalInput").ap()
    identb_d = nc.dram_tensor("identb", [128, 128], bf16,
                              kind="ExternalInput").ap()
    pos_out = nc.dram_tensor("pos_out", [b_sz, 1024], f32,
                             kind="ExternalOutput").ap()
    vel_out = nc.dram_tensor("vel_out", [b_sz, 1024], f32,
                             kind="ExternalOutput").ap()

    with tile.TileContext(nc) as tc, ExitStack() as ctx:
        const = ctx.enter_context(tc.tile_pool(name="const", bufs=1))
        ztp = ctx.enter_context(tc.tile_pool(name="ztp", bufs=1))
        zfp = ctx.enter_context(tc.tile_pool(name="zfp", bufs=3))
        zbp = ctx.enter_context(tc.tile_pool(name="zbp", bufs=3))
        stposp = ctx.enter_context(tc.tile_pool(name="stposp", bufs=3))
        stvelp = ctx.enter_context(tc.tile_pool(name="stvelp", bufs=3))
        planep = ctx.enter_context(tc.tile_pool(name="planep", bufs=2))
        ps_main = ctx.enter_context(tc.tile_pool(name="ps_main", bufs=4,
                                                 space="PSUM"))
        ps_tr = ctx.enter_context(tc.tile_pool(name="ps_tr", bufs=2,
                                               space="PSUM"))
        ps_warm = ctx.enter_context(tc.tile_pool(name="ps_warm", bufs=1,
                                                 space="PSUM"))

        # ---- constants -> SBUF.  identb first (unblocks PE warm-up). ----
        identb_sb = const.tile([128, 128], bf16, name="identb_sb", tag="identb")
        nc.sync.dma_start(identb_sb[:], identb_d)
        p0sb = const.tile([128, n_bt, 2], f32, name="p0sb", tag="p0sb")
        nc.sync.dma_start(p0sb[:, :, :],
                          p0_in.rearrange("(a b) c -> b a c", a=n_bt))
        wmain_sb = [[const.tile([128, 512], bf16, name=f"wm_{ms}_{h}",
                                tag=f"wm{ms}{h}")
                     for h in range(2)] for ms in range(2)]
        for ms in range(2):
            for h in range(2):
                nc.scalar.dma_start(wmain_sb[ms][h][:], wmain_d[ms][h])
        bvrow_sb = const.tile([128, 128, 1], f32, name="bvrow_sb", tag="bvrow")
        avrow_sb = const.tile([128, 128, 1], f32, name="avrow_sb", tag="avrow")
        nc.scalar.dma_start(bvrow_sb[:, :, :], bvrow_d)
        nc.scalar.dma_start(avrow_sb[:, :, :], avrow_d)

        # ---- PE warm-up: dummy matmuls while input DMAs are in flight,
        # so the HAM clock gate reaches 2.4 GHz before real work ----
        warm_ps = ps_warm.tile([128, 128], f32, name="warm_ps", tag="warm")
        NWARM = 24
        for wi in range(NWARM):
            nc.tensor.matmul(warm_ps[:], identb_sb[:], identb_sb[:],
                             start=(wi == 0), stop=(wi == NWARM - 1))

        # ztm[m]: time-major data, [128 (s,c)] x [h, b]; partition q of
        # half h <-> (s = 64 m' + q//2, c = q&1) matching wmain rows.
        zt = [ztp.tile([128, 2, b_sz], bf16, name=f"zt_{m}", tag=f"zt{m}")
              for m in range(NCH)]

        def load_z(j):
            zf = zfp.tile([128, 1024], f32, name=f"zf_{j}", tag="zf")
            nc.sync.dma_start(zf[:], z_in[128 * j:128 * (j + 1), :])
            return zf

        def cast_z(j, zf):
            zb = zbp.tile([128, 1024], bf16, name=f"zb_{j}", tag="zb")
            nc.vector.tensor_copy(zb[:], zf[:])
            return zb

        def transp(j, zb):
            """8 PE transposes of tile j, batched 4 per PSUM tile; copies
            into zt[m] split between DVE and ACT."""
            bsl = slice(128 * j, 128 * (j + 1))
            for q in range(2):
                tp = ps_tr.tile([128, 512], bf16, name=f"tp_{j}_{q}", tag="tp")
                for i in range(4):
                    k = 4 * q + i
                    nc.tensor.matmul(tp[:, 128 * i:128 * (i + 1)],
                                     zb[:, 128 * k:128 * (k + 1)],
                                     identb_sb[:], is_transpose=True)
                lo = tp[:, 0:256].rearrange("p (h f) -> p h f", h=2)
                hi = tp[:, 256:512].rearrange("p (h f) -> p h f", h=2)
                nc.vector.tensor_copy(zt[2 * q][:, :, bsl], lo)
                nc.vector.tensor_copy(zt[2 * q + 1][:, :, bsl], hi)

        def mains(j):
            """Chunk matmuls + carry-plane evacuation for tile j."""
            bsl = slice(128 * j, 128 * (j + 1))
            stpos = stposp.tile([128, 512, 2], f32, name=f"stpos_{j}",
                                tag="stpos")
            stvel = stvelp.tile([128, 512, 2], f32, name=f"stvel_{j}",
                                tag="stvel")
            for m in range(NCH):
                ms = min(m, 1)
                out_ps = ps_main.tile([128, 256, 2], f32, name=f"out_{j}_{m}",
                                      tag="out")
                nc.tensor.matmul(out_ps[:, :, :], zt[m][:, 0:1, bsl],
                                 wmain_sb[ms][0][:], start=True, stop=False)
                nc.tensor.matmul(out_ps[:, :, :], zt[m][:, 1:2, bsl],
                                 wmain_sb[ms][1][:], start=False, stop=True)

                csl = slice(128 * m, 128 * (m + 1))
                if m == 0:
                    # carry-in is (p0, 0): pos += p0_c; vel copies through
                    for c in range(2):
                        nc.vector.tensor_scalar(
                            stpos[:, 0:128, c:c + 1],
                            out_ps[:, 0:128, c:c + 1],
                            p0sb[:, j:j + 1, c:c + 1], None, ADD)
                    nc.scalar.copy(stvel[:, 0:128, :], out_ps[:, 128:256, :])
                else:
                    # pos carry plane p_in + Bv v_in on ACT (per-partition
                    # scale/bias); vel carry fused into the evacuation via
                    # DVE scalar_tensor_tensor: (Av * v_in) + psum.
                    psl = slice(128 * m - 1, 128 * m)
                    plp = planep.tile([128, 128, 2], f32, name=f"plp_{j}_{m}",
                                      tag="plp")
                    for c in range(2):
                        vin = stvel[:, psl, c:c + 1]
                        pin = stpos[:, psl, c:c + 1]
                        nc.scalar.activation(
                            plp[:, :, c:c + 1], bvrow_sb[:, :, :], IDENT,
                            bias=pin, scale=vin)
                        nc.vector.scalar_tensor_tensor(
                            stvel[:, csl, c:c + 1], avrow_sb[:, :, :],
                            vin, out_ps[:, 128:256, c:c + 1], MULT, ADD)
                    nc.vector.tensor_add(stpos[:, csl, :],
                                         out_ps[:, 0:128, :], plp[:, :, :])
            nc.scalar.dma_start(pos_out[bsl, :], stpos[:, :, :])
            nc.gpsimd.dma_start(vel_out[bsl, :], stvel[:, :, :])

        # ---- software-pipelined main loop ----
        zf = {0: load_z(0), 1: load_z(1)}
        zb = {0: cast_z(0, zf[0])}
        transp(0, zb[0])
        for j in range(n_bt):
            if j + 2 < n_bt:
                zf[j + 2] = load_z(j + 2)
            if j + 1 < n_bt:
                zb[j + 1] = cast_z(j + 1, zf[j + 1])
                transp(j + 1, zb[j + 1])
            mains(j)

    nc.compile()
    return nc


# ----------------------------------------------------------------------------
# Host entry point
# ----------------------------------------------------------------------------

_CACHE = {}

# test-harness knobs (ignored in normal use)
PROFILE = False
LAST_RESULT = None


def _get_nc(n_bt):
    if n_bt not in _CACHE:
        _CACHE[n_bt] = build_nc(n_bt)
    return _CACHE[n_bt]


def kernel(pred_vel, dt, p0, q_pos, q_vel, r_vel):
    from concourse.bass_utils import run_bass_kernel_spmd

    z = np.ascontiguousarray(np.asarray(pred_vel, dtype=np.float32))
    p0 = np.ascontiguousarray(np.asarray(p0, dtype=np.float32))
    assert z.shape == (B_FULL, T, 2) and p0.shape == (B_FULL, 2)

    weights = build_weights(dt, q_pos, q_vel, r_vel)
    nc = _get_nc(B_CORE // 128)

    in_maps = []
    for i in range(N_CORES):
        sl = slice(i * B_CORE, (i + 1) * B_CORE)
        m = {"z_in": z[sl].reshape(B_CORE, 2 * T),
             "p0_in": p0[sl]}
        m.update(weights)
        in_maps.append(m)

    res = run_bass_kernel_spmd(nc, in_maps, core_ids=list(range(N_CORES)),
                               trace=PROFILE)
    global LAST_RESULT
    LAST_RESULT = res
    pos = np.concatenate([r["pos_out"].reshape(B_CORE, T, 2)
                          for r in res.results], axis=0)
    vel = np.concatenate([r["vel_out"].reshape(B_CORE, T, 2)
                          for r in res.results], axis=0)
    return pos, vel


# revision 15
# speedup vs baseline: 1.0805x; 1.0805x over previous
"""Trainium2 Bass kernel for nn_BatchDifferentiableKF.

Problem: batched 4-state Kalman filter, B=16384 batch rows, T=512 steps,
state [px, py, vx, vy], measurements = predicted velocities (B, T, 2).

Key structure exploited:
  * The covariance/gain recursion is data-independent (P0 = I for every
    batch row), so the Kalman gains k_p[t], k_v[t] are a fixed schedule
    computed on host in float64.
  * x/y components decouple into two identical scalar filters:
        p_t = p_{t-1} + g[t] v_{t-1} + k_p[t] z_t      (g = dt - k_p)
        v_t = a[t] v_{t-1} + k_v[t] z_t                (a = 1 - k_v)
    i.e. the whole filter is LINEAR in (z, p0).
  * Chunk T into 4 x 128. Within a chunk the map z -> outputs is a dense
    lower-triangular 128x128 matrix pair (Wp, Wv); chunks 1..3 share
    identical (steady-state) weights. The cross-chunk carry enters as
        p_out[i] += p_in + Bv[i] v_in ;  v_out[i] += Av[i] v_in
    which is applied in BATCH-MAJOR form during PSUM evacuation: the
    carry scalars (p_in, v_in) are the t=127 columns of the previous
    chunk's staged output, used as per-partition scale/bias operands of
    ALU/ACT ops against constant Bv/Av row-broadcast tiles. No
    transposed carry state, no carry matmuls.

Device algorithm per core (2048 batch rows, 16 batch-tiles):
  1. DMA z fp32 tile, DVE-cast to bf16, PE-transpose (8x 128x128) into
     time-major tiles ztm[m] [128 (s,c) x 2, b], batched 4-per-PSUM.
  2. Per (tile j, chunk m): 2 PE matmuls (bf16 data stationary, weights
     moving) produce out[128 b, 512 (t,c,pv)] in PSUM; DVE adds the
     carry planes (built on GpSimd/Scalar from the previous chunk's
     staged columns) while evacuating to SBUF stage tiles.
  3. Per-tile output DMAs: pos on the scalar HWDGE ring, vel on the
     gpsimd SWDGE ring; z loads on the sync HWDGE ring.

Sharding: embarrassingly parallel over batch across the 8 cores.
"""

import numpy as np
import ml_dtypes

B_FULL = 16384
T = 512
C = 128          # chunk length
NCH = T // C     # 4 chunks
N_CORES = 8
B_CORE = B_FULL // N_CORES   # 2048


# ----------------------------------------------------------------------------
# Host-side weight construction (float64)
# ----------------------------------------------------------------------------

def _gains(dt, q_pos, q_vel, r_vel, n):
    """Gain schedule k_p[t], k_v[t] of the decoupled scalar filter, P0=I."""
    dt = float(np.float32(dt))
    r = float(np.float32(r_vel)) + float(np.float32(1e-6))
    qp = float(np.float32(q_pos))
    qv = float(np.float32(q_vel))
    Ppp, Ppv, Pvv = 1.0, 0.0, 1.0
    k_p = np.zeros(n)
    k_v = np.zeros(n)
    for t in range(n):
        Ppv_ = Ppv + dt * Pvv
        Ppp_ = Ppp + 2.0 * dt * Ppv + dt * dt * Pvv + qp
        Pvv_ = Pvv + qv
        S = Pvv_ + r
        k_p[t] = Ppv_ / S
        k_v[t] = Pvv_ / S
        Ppp = Ppp_ - k_p[t] * Ppv_
        Ppv = Ppv_ - k_p[t] * Pvv_
        Pvv = Pvv_ - k_v[t] * Pvv_
    return k_p, k_v


def _chunk_maps(k_p, k_v, dt):
    """Per-chunk affine maps: (p_in, v_in, z[0..C-1]) -> (p[0..C-1], v[..]).

    p_out[i] = p_in + Bv[m][i] v_in + sum_j Wp[m][i,j] z[j]
    v_out[i] =        Av[m][i] v_in + sum_j Wv[m][i,j] z[j]
    """
    g = dt - k_p
    a = 1.0 - k_v
    Wp = np.zeros((NCH, C, C))
    Wv = np.zeros((NCH, C, C))
    Av = np.zeros((NCH, C))
    Bv = np.zeros((NCH, C))
    for m in range(NCH):
        pcoef = np.zeros(C + 1)
        vcoef = np.zeros(C + 1)
        vcoef[0] = 1.0
        for i in range(C):
            t = m * C + i
            pcoef = pcoef + g[t] * vcoef
            pcoef[1 + i] += k_p[t]
            vcoef = a[t] * vcoef
            vcoef[1 + i] += k_v[t]
            Bv[m, i] = pcoef[0]
            Wp[m, i] = pcoef[1:]
            Av[m, i] = vcoef[0]
            Wv[m, i] = vcoef[1:]
    return Wp, Wv, Av, Bv


def build_weights(dt, q_pos, q_vel, r_vel):
    """Device constant tensors. Layouts:

    partition index q of a transposed-data tile <-> (j_local = 64h + q//2,
    c = q&1) for tile half h; output free index f = pv*256 + t*2 + c'.
    bvrow/avrow: [128, 128] f32 with identical rows Bv[1]/Av[1] (steady
    chunk map), consumed with per-partition scale/bias carries.
    """
    dtf = float(np.float32(dt))
    k_p, k_v = _gains(dt, q_pos, q_vel, r_vel, T)
    Wp, Wv, Av, Bv = _chunk_maps(k_p, k_v, dtf)

    bf16 = ml_dtypes.bfloat16
    out = {}
    for mset in range(2):
        mc = mset  # chunk-map index (chunk 0, or steady chunk 1)
        for h in range(2):
            w = np.zeros((128, 512))
            for q in range(128):
                j = 64 * h + q // 2
                c = q & 1
                # f = pv*256 + t*2 + c ; delta_{c,c'} keeps only c'==c
                w[q, 0 * 256 + 2 * np.arange(C) + c] = Wp[mc, :, j]
                w[q, 1 * 256 + 2 * np.arange(C) + c] = Wv[mc, :, j]
            out[f"wmain_{mset}_{h}"] = w.astype(bf16)
    out["bvrow"] = np.broadcast_to(Bv[1], (128, C)).astype(np.float32).copy()
    out["avrow"] = np.broadcast_to(Av[1], (128, C)).astype(np.float32).copy()
    out["identb"] = np.eye(128, dtype=bf16)
    return out


# ----------------------------------------------------------------------------
# Bass kernel
# ----------------------------------------------------------------------------

def build_nc(n_bt):
    """Build the Bass program for one core processing n_bt*128 batch rows."""
    import concourse.bass as bass
    import concourse.tile as tile
    from concourse import bacc, mybir
    from contextlib import ExitStack

    f32 = mybir.dt.float32
    bf16 = mybir.dt.bfloat16
    ADD = mybir.AluOpType.add
    MULT = mybir.AluOpType.mult
    IDENT = mybir.ActivationFunctionType.Identity

    b_sz = n_bt * 128
    nc = bacc.Bacc("TRN2", target_bir_lowering=False, debug=False)

    z_in = nc.dram_tensor("z_in", [b_sz, 1024], f32, kind="ExternalInput").ap()
    p0_in = nc.dram_tensor("p0_in", [b_sz, 2], f32, kind="ExternalInput").ap()
    wmain_d = [[nc.dram_tensor(f"wmain_{ms}_{h}", [128, 512], bf16,
                               kind="ExternalInput").ap()
                for h in range(2)] for ms in range(2)]
    bvrow_d = nc.dram_tensor("bvrow", [128, 128], f32, kind="ExternalInput").ap()
    avrow_d = nc.dram_tensor("avrow", [128, 128], f32, kind="ExternalInput").ap()
    identb_d = nc.dram_tensor("identb", [128, 128], bf16,
                              kind="ExternalInput").ap()
    pos_out = nc.dram_tensor("pos_out", [b_sz, 1024], f32,
                             kind="ExternalOutput").ap()
    vel_out = nc.dram_tensor("vel_out", [b_sz, 1024], f32,
                             kind="ExternalOutput").ap()

    with tile.TileContext(nc) as tc, ExitStack() as ctx:
        const = ctx.enter_context(tc.tile_pool(name="const", bufs=1))
        ztp = ctx.enter_context(tc.tile_pool(name="ztp", bufs=1))
        zfp = ctx.enter_context(tc.tile_pool(name="zfp", bufs=3))
        zbp = ctx.enter_context(tc.tile_pool(name="zbp", bufs=3))
        stposp = ctx.enter_context(tc.tile_pool(name="stposp", bufs=3))
        stvelp = ctx.enter_context(tc.tile_pool(name="stvelp", bufs=3))
        planep = ctx.enter_context(tc.tile_pool(name="planep", bufs=2))
        ps_main = ctx.enter_context(tc.tile_pool(name="ps_main", bufs=4,
                                                 space="PSUM"))
        ps_tr = ctx.enter_context(tc.tile_pool(name="ps_tr", bufs=2,
                                               space="PSUM"))
        ps_warm = ctx.enter_context(tc.tile_pool(name="ps_warm", bufs=1,
                                                 space="PSUM"))

        # ---- constants -> SBUF.  identb first (unblocks PE warm-up). ----
        identb_sb = const.tile([128, 128], bf16, name="identb_sb", tag="identb")
        nc.sync.dma_start(identb_sb[:], identb_d)
        p0sb = const.tile([128, n_bt, 2], f32, name="p0sb", tag="p0sb")
        nc.sync.dma_start(p0sb[:, :, :],
                          p0_in.rearrange("(a b) c -> b a c", a=n_bt))
        wmain_sb = [[const.tile([128, 512], bf16, name=f"wm_{ms}_{h}",
                                tag=f"wm{ms}{h}")
                     for h in range(2)] for ms in range(2)]
        for ms in range(2):
            for h in range(2):
                nc.scalar.dma_start(wmain_sb[ms][h][:], wmain_d[ms][h])
        bvrow_sb = const.tile([128, 128, 1], f32, name="bvrow_sb", tag="bvrow")
        avrow_sb = const.tile([128, 128, 1], f32, name="avrow_sb", tag="avrow")
        nc.scalar.dma_start(bvrow_sb[:, :, :], bvrow_d)
        nc.scalar.dma_start(avrow_sb[:, :, :], avrow_d)

        # ---- PE warm-up: dummy matmuls while input DMAs are in flight,
        # so the HAM clock gate reaches 2.4 GHz before real work ----
        warm_ps = ps_warm.tile([128, 128], f32, name="warm_ps", tag="warm")
        NWARM = 24
        for wi in range(NWARM):
            nc.tensor.matmul(warm_ps[:], identb_sb[:], identb_sb[:],
                             start=(wi == 0), stop=(wi == NWARM - 1))

        # ztm[m]: time-major data, [128 (s,c)] x [h, b]; partition q of
        # half h <-> (s = 64 m' + q//2, c = q&1) matching wmain rows.
        zt = [ztp.tile([128, 2, b_sz], bf16, name=f"zt_{m}", tag=f"zt{m}")
              for m in range(NCH)]

        def load_z(j):
            zf = zfp.tile([128, 1024], f32, name=f"zf_{j}", tag="zf")
            nc.sync.dma_start(zf[:], z_in[128 * j:128 * (j + 1), :])
            return zf

        def cast_z(j, zf):
            zb = zbp.tile([128, 1024], bf16, name=f"zb_{j}", tag="zb")
            nc.vector.tensor_copy(zb[:], zf[:])
            return zb

        def transp(j, zb):
            """8 PE transposes of tile j, batched 4 per PSUM tile; copies
            into zt[m] split between DVE and ACT."""
            bsl = slice(128 * j, 128 * (j + 1))
            for q in range(2):
                tp = ps_tr.tile([128, 512], bf16, name=f"tp_{j}_{q}", tag="tp")
                for i in range(4):
                    k = 4 * q + i
                    nc.tensor.matmul(tp[:, 128 * i:128 * (i + 1)],
                                     zb[:, 128 * k:128 * (k + 1)],
                                     identb_sb[:], is_transpose=True)
                lo = tp[:, 0:256].rearrange("p (h f) -> p h f", h=2)
                hi = tp[:, 256:512].rearrange("p (h f) -> p h f", h=2)
                nc.vector.tensor_copy(zt[2 * q][:, :, bsl], lo)
                nc.scalar.copy(zt[2 * q + 1][:, :, bsl], hi)

        def mains(j):
            """Chunk matmuls + carry-plane evacuation for tile j."""
            bsl = slice(128 * j, 128 * (j + 1))
            stpos = stposp.tile([128, 512, 2], f32, name=f"stpos_{j}",
                                tag="stpos")
            stvel = stvelp.tile([128, 512, 2], f32, name=f"stvel_{j}",
                                tag="stvel")
            for m in range(NCH):
                ms = min(m, 1)
                out_ps = ps_main.tile([128, 256, 2], f32, name=f"out_{j}_{m}",
                                      tag="out")
                nc.tensor.matmul(out_ps[:, :, :], zt[m][:, 0:1, bsl],
                                 wmain_sb[ms][0][:], start=True, stop=False)
                nc.tensor.matmul(out_ps[:, :, :], zt[m][:, 1:2, bsl],
                                 wmain_sb[ms][1][:], start=False, stop=True)

                csl = slice(128 * m, 128 * (m + 1))
                if m == 0:
                    # carry-in is (p0, 0): pos += p0_c; vel copies through
                    for c in range(2):
                        nc.vector.tensor_scalar(
                            stpos[:, 0:128, c:c + 1],
                            out_ps[:, 0:128, c:c + 1],
                            p0sb[:, j:j + 1, c:c + 1], None, ADD)
                    nc.scalar.copy(stvel[:, 0:128, :], out_ps[:, 128:256, :])
                else:
                    # pos carry plane p_in + Bv v_in on ACT (per-partition
                    # scale/bias); vel carry fused into the evacuation via
                    # DVE scalar_tensor_tensor: (Av * v_in) + psum.
                    psl = slice(128 * m - 1, 128 * m)
                    plp = planep.tile([128, 128, 2], f32, name=f"plp_{j}_{m}",
                                      tag="plp")
                    for c in range(2):
                        vin = stvel[:, psl, c:c + 1]
                        pin = stpos[:, psl, c:c + 1]
                        nc.scalar.activation(
                            plp[:, :, c:c + 1], bvrow_sb[:, :, :], IDENT,
                            bias=pin, scale=vin)
                        nc.vector.scalar_tensor_tensor(
                            stvel[:, csl, c:c + 1], avrow_sb[:, :, :],
                            vin, out_ps[:, 128:256, c:c + 1], MULT, ADD)
                    nc.vector.tensor_add(stpos[:, csl, :],
                                         out_ps[:, 0:128, :], plp[:, :, :])
            nc.scalar.dma_start(pos_out[bsl, :], stpos[:, :, :])
            nc.gpsimd.dma_start(vel_out[bsl, :], stvel[:, :, :])

        # ---- software-pipelined main loop ----
        zf = {0: load_z(0), 1: load_z(1)}
        zb = {0: cast_z(0, zf[0])}
        transp(0, zb[0])
        for j in range(n_bt):
            if j + 2 < n_bt:
                zf[j + 2] = load_z(j + 2)
            if j + 1 < n_bt:
                zb[j + 1] = cast_z(j + 1, zf[j + 1])
                transp(j + 1, zb[j + 1])
            mains(j)

    nc.compile()
    return nc


# ----------------------------------------------------------------------------
# Host entry point
# ----------------------------------------------------------------------------

_CACHE = {}

# test-harness knobs (ignored in normal use)
PROFILE = False
LAST_RESULT = None


def _get_nc(n_bt):
    if n_bt not in _CACHE:
        _CACHE[n_bt] = build_nc(n_bt)
    return _CACHE[n_bt]


def kernel(pred_vel, dt, p0, q_pos, q_vel, r_vel):
    from concourse.bass_utils import run_bass_kernel_spmd

    z = np.ascontiguousarray(np.asarray(pred_vel, dtype=np.float32))
    p0 = np.ascontiguousarray(np.asarray(p0, dtype=np.float32))
    assert z.shape == (B_FULL, T, 2) and p0.shape == (B_FULL, 2)

    weights = build_weights(dt, q_pos, q_vel, r_vel)
    nc = _get_nc(B_CORE // 128)

    in_maps = []
    for i in range(N_CORES):
        sl = slice(i * B_CORE, (i + 1) * B_CORE)
        m = {"z_in": z[sl].reshape(B_CORE, 2 * T),
             "p0_in": p0[sl]}
        m.update(weights)
        in_maps.append(m)

    res = run_bass_kernel_spmd(nc, in_maps, core_ids=list(range(N_CORES)),
                               trace=PROFILE)
    global LAST_RESULT
    LAST_RESULT = res
    pos = np.concatenate([r["pos_out"].reshape(B_CORE, T, 2)
                          for r in res.results], axis=0)
    vel = np.concatenate([r["vel_out"].reshape(B_CORE, T, 2)
                          for r in res.results], axis=0)
    return pos, vel


# revision 16
# speedup vs baseline: 1.1213x; 1.0377x over previous
"""Trainium2 Bass kernel for nn_BatchDifferentiableKF.

Problem: batched 4-state Kalman filter, B=16384 batch rows, T=512 steps,
state [px, py, vx, vy], measurements = predicted velocities (B, T, 2).

Key structure exploited:
  * The covariance/gain recursion is data-independent (P0 = I for every
    batch row), so the Kalman gains k_p[t], k_v[t] are a fixed schedule
    computed on host in float64.
  * x/y components decouple into two identical scalar filters:
        p_t = p_{t-1} + g[t] v_{t-1} + k_p[t] z_t      (g = dt - k_p)
        v_t = a[t] v_{t-1} + k_v[t] z_t                (a = 1 - k_v)
    i.e. the whole filter is LINEAR in (z, p0).
  * Chunk T into 4 x 128. Within a chunk the map z -> outputs is a dense
    lower-triangular 128x128 matrix pair (Wp, Wv); chunks 1..3 share
    identical (steady-state) weights. The cross-chunk carry enters as
        p_out[i] += p_in + Bv[i] v_in ;  v_out[i] += Av[i] v_in
    which is applied in BATCH-MAJOR form during PSUM evacuation: the
    carry scalars (p_in, v_in) are the t=127 columns of the previous
    chunk's staged output, used as per-partition scale/bias operands of
    ALU/ACT ops against constant Bv/Av row-broadcast tiles. No
    transposed carry state, no carry matmuls.

Device algorithm per core (2048 batch rows, 16 batch-tiles):
  1. DMA z fp32 tile, DVE-cast to bf16, PE-transpose (8x 128x128) into
     time-major tiles ztm[m] [128 (s,c) x 2, b], batched 4-per-PSUM.
  2. Per (tile j, chunk m): 2 PE matmuls (bf16 data stationary, weights
     moving) produce out[128 b, 512 (t,c,pv)] in PSUM; DVE adds the
     carry planes (built on GpSimd/Scalar from the previous chunk's
     staged columns) while evacuating to SBUF stage tiles.
  3. Per-tile output DMAs: pos on the scalar HWDGE ring, vel on the
     gpsimd SWDGE ring; z loads on the sync HWDGE ring.

Sharding: embarrassingly parallel over batch across the 8 cores.
"""

import numpy as np
import ml_dtypes

B_FULL = 16384
T = 512
C = 128          # chunk length
NCH = T // C     # 4 chunks
N_CORES = 8
B_CORE = B_FULL // N_CORES   # 2048


# ----------------------------------------------------------------------------
# Host-side weight construction (float64)
# ----------------------------------------------------------------------------

def _gains(dt, q_pos, q_vel, r_vel, n):
    """Gain schedule k_p[t], k_v[t] of the decoupled scalar filter, P0=I."""
    dt = float(np.float32(dt))
    r = float(np.float32(r_vel)) + float(np.float32(1e-6))
    qp = float(np.float32(q_pos))
    qv = float(np.float32(q_vel))
    Ppp, Ppv, Pvv = 1.0, 0.0, 1.0
    k_p = np.zeros(n)
    k_v = np.zeros(n)
    for t in range(n):
        Ppv_ = Ppv + dt * Pvv
        Ppp_ = Ppp + 2.0 * dt * Ppv + dt * dt * Pvv + qp
        Pvv_ = Pvv + qv
        S = Pvv_ + r
        k_p[t] = Ppv_ / S
        k_v[t] = Pvv_ / S
        Ppp = Ppp_ - k_p[t] * Ppv_
        Ppv = Ppv_ - k_p[t] * Pvv_
        Pvv = Pvv_ - k_v[t] * Pvv_
    return k_p, k_v


def _chunk_maps(k_p, k_v, dt):
    """Per-chunk affine maps: (p_in, v_in, z[0..C-1]) -> (p[0..C-1], v[..]).

    p_out[i] = p_in + Bv[m][i] v_in + sum_j Wp[m][i,j] z[j]
    v_out[i] =        Av[m][i] v_in + sum_j Wv[m][i,j] z[j]
    """
    g = dt - k_p
    a = 1.0 - k_v
    Wp = np.zeros((NCH, C, C))
    Wv = np.zeros((NCH, C, C))
    Av = np.zeros((NCH, C))
    Bv = np.zeros((NCH, C))
    for m in range(NCH):
        pcoef = np.zeros(C + 1)
        vcoef = np.zeros(C + 1)
        vcoef[0] = 1.0
        for i in range(C):
            t = m * C + i
            pcoef = pcoef + g[t] * vcoef
            pcoef[1 + i] += k_p[t]
            vcoef = a[t] * vcoef
            vcoef[1 + i] += k_v[t]
            Bv[m, i] = pcoef[0]
            Wp[m, i] = pcoef[1:]
            Av[m, i] = vcoef[0]
            Wv[m, i] = vcoef[1:]
    return Wp, Wv, Av, Bv


def build_weights(dt, q_pos, q_vel, r_vel):
    """Device constant tensors. Layouts:

    partition index q of a transposed-data tile <-> (j_local = 64h + q//2,
    c = q&1) for tile half h; output free index f = pv*256 + t*2 + c'.
    bvrow/avrow: [128, 128] f32 with identical rows Bv[1]/Av[1] (steady
    chunk map), consumed with per-partition scale/bias carries.
    """
    dtf = float(np.float32(dt))
    k_p, k_v = _gains(dt, q_pos, q_vel, r_vel, T)
    Wp, Wv, Av, Bv = _chunk_maps(k_p, k_v, dtf)

    bf16 = ml_dtypes.bfloat16
    out = {}
    for mset in range(2):
        mc = mset  # chunk-map index (chunk 0, or steady chunk 1)
        for h in range(2):
            w = np.zeros((128, 512))
            for q in range(128):
                j = 64 * h + q // 2
                c = q & 1
                # f = pv*256 + t*2 + c ; delta_{c,c'} keeps only c'==c
                w[q, 0 * 256 + 2 * np.arange(C) + c] = Wp[mc, :, j]
                w[q, 1 * 256 + 2 * np.arange(C) + c] = Wv[mc, :, j]
            out[f"wmain_{mset}_{h}"] = w.astype(bf16)
    out["bvrow"] = np.broadcast_to(Bv[1], (128, C)).astype(np.float32).copy()
    out["avrow"] = np.broadcast_to(Av[1], (128, C)).astype(np.float32).copy()
    out["identb"] = np.eye(128, dtype=bf16)
    return out


# ----------------------------------------------------------------------------
# Bass kernel
# ----------------------------------------------------------------------------

def build_nc(n_bt):
    """Build the Bass program for one core processing n_bt*128 batch rows."""
    import concourse.bass as bass
    import concourse.tile as tile
    from concourse import bacc, mybir
    from contextlib import ExitStack

    f32 = mybir.dt.float32
    bf16 = mybir.dt.bfloat16
    ADD = mybir.AluOpType.add
    MULT = mybir.AluOpType.mult
    IDENT = mybir.ActivationFunctionType.Identity

    b_sz = n_bt * 128
    nc = bacc.Bacc("TRN2", target_bir_lowering=False, debug=False)

    z_in = nc.dram_tensor("z_in", [b_sz, 1024], f32, kind="ExternalInput").ap()
    p0_in = nc.dram_tensor("p0_in", [b_sz, 2], f32, kind="ExternalInput").ap()
    wmain_d = [[nc.dram_tensor(f"wmain_{ms}_{h}", [128, 512], bf16,
                               kind="ExternalInput").ap()
                for h in range(2)] for ms in range(2)]
    bvrow_d = nc.dram_tensor("bvrow", [128, 128], f32, kind="ExternalInput").ap()
    avrow_d = nc.dram_tensor("avrow", [128, 128], f32, kind="ExternalInput").ap()
    identb_d = nc.dram_tensor("identb", [128, 128], bf16,
                              kind="ExternalInput").ap()
    pos_out = nc.dram_tensor("pos_out", [b_sz, 1024], f32,
                             kind="ExternalOutput").ap()
    vel_out = nc.dram_tensor("vel_out", [b_sz, 1024], f32,
                             kind="ExternalOutput").ap()

    with tile.TileContext(nc) as tc, ExitStack() as ctx:
        const = ctx.enter_context(tc.tile_pool(name="const", bufs=1))
        ztp = ctx.enter_context(tc.tile_pool(name="ztp", bufs=1))
        zfp = ctx.enter_context(tc.tile_pool(name="zfp", bufs=3))
        zbp = ctx.enter_context(tc.tile_pool(name="zbp", bufs=3))
        stposp = ctx.enter_context(tc.tile_pool(name="stposp", bufs=3))
        stvelp = ctx.enter_context(tc.tile_pool(name="stvelp", bufs=3))
        planep = ctx.enter_context(tc.tile_pool(name="planep", bufs=2))
        ps_main = ctx.enter_context(tc.tile_pool(name="ps_main", bufs=4,
                                                 space="PSUM"))
        ps_tr = ctx.enter_context(tc.tile_pool(name="ps_tr", bufs=2,
                                               space="PSUM"))
        ps_warm = ctx.enter_context(tc.tile_pool(name="ps_warm", bufs=1,
                                                 space="PSUM"))

        # ---- constants -> SBUF.  identb first (unblocks PE warm-up). ----
        identb_sb = const.tile([128, 128], bf16, name="identb_sb", tag="identb")
        nc.sync.dma_start(identb_sb[:], identb_d)
        p0sb = const.tile([128, n_bt, 2], f32, name="p0sb", tag="p0sb")
        nc.sync.dma_start(p0sb[:, :, :],
                          p0_in.rearrange("(a b) c -> b a c", a=n_bt))
        wmain_sb = [[const.tile([128, 512], bf16, name=f"wm_{ms}_{h}",
                                tag=f"wm{ms}{h}")
                     for h in range(2)] for ms in range(2)]
        for ms in range(2):
            for h in range(2):
                nc.scalar.dma_start(wmain_sb[ms][h][:], wmain_d[ms][h])
        bvrow_sb = const.tile([128, 128, 1], f32, name="bvrow_sb", tag="bvrow")
        avrow_sb = const.tile([128, 128, 1], f32, name="avrow_sb", tag="avrow")
        nc.scalar.dma_start(bvrow_sb[:, :, :], bvrow_d)
        nc.scalar.dma_start(avrow_sb[:, :, :], avrow_d)

        # ---- PE warm-up: dummy matmuls while input DMAs are in flight,
        # so the HAM clock gate reaches 2.4 GHz before real work ----
        warm_ps = ps_warm.tile([128, 128], f32, name="warm_ps", tag="warm")
        NWARM = 24
        for wi in range(NWARM):
            nc.tensor.matmul(warm_ps[:], identb_sb[:], identb_sb[:],
                             start=(wi == 0), stop=(wi == NWARM - 1))

        # ztm[m]: time-major data, [128 (s,c)] x [h, b]; partition q of
        # half h <-> (s = 64 m' + q//2, c = q&1) matching wmain rows.
        zt = [ztp.tile([128, 2, b_sz], bf16, name=f"zt_{m}", tag=f"zt{m}")
              for m in range(NCH)]

        def load_z(j):
            zf = zfp.tile([128, 1024], f32, name=f"zf_{j}", tag="zf")
            nc.sync.dma_start(zf[:], z_in[128 * j:128 * (j + 1), :])
            return zf

        def cast_z(j, zf):
            zb = zbp.tile([128, 1024], bf16, name=f"zb_{j}", tag="zb")
            nc.vector.tensor_copy(zb[:], zf[:])
            return zb

        def transp(j, zb):
            """8 PE transposes of tile j, batched 4 per PSUM tile; copies
            into zt[m] split between DVE and ACT."""
            bsl = slice(128 * j, 128 * (j + 1))
            for q in range(2):
                tp = ps_tr.tile([128, 512], bf16, name=f"tp_{j}_{q}", tag="tp")
                for i in range(4):
                    k = 4 * q + i
                    nc.tensor.matmul(tp[:, 128 * i:128 * (i + 1)],
                                     zb[:, 128 * k:128 * (k + 1)],
                                     identb_sb[:], is_transpose=True)
                lo = tp[:, 0:256].rearrange("p (h f) -> p h f", h=2)
                hi = tp[:, 256:512].rearrange("p (h f) -> p h f", h=2)
                nc.vector.tensor_copy(zt[2 * q][:, :, bsl], lo)
                nc.scalar.copy(zt[2 * q + 1][:, :, bsl], hi)

        def mains(j):
            """Chunk matmuls + carry-plane evacuation for tile j."""
            bsl = slice(128 * j, 128 * (j + 1))
            stpos = stposp.tile([128, 512, 2], f32, name=f"stpos_{j}",
                                tag="stpos")
            stvel = stvelp.tile([128, 512, 2], f32, name=f"stvel_{j}",
                                tag="stvel")
            for m in range(NCH):
                ms = min(m, 1)
                out_ps = ps_main.tile([128, 256, 2], f32, name=f"out_{j}_{m}",
                                      tag="out")
                nc.tensor.matmul(out_ps[:, :, :], zt[m][:, 0:1, bsl],
                                 wmain_sb[ms][0][:], start=True, stop=False)
                nc.tensor.matmul(out_ps[:, :, :], zt[m][:, 1:2, bsl],
                                 wmain_sb[ms][1][:], start=False, stop=True)

                csl = slice(128 * m, 128 * (m + 1))
                if m == 0:
                    # carry-in is (p0, 0): pos += p0_c; vel copies through
                    for c in range(2):
                        nc.vector.tensor_scalar(
                            stpos[:, 0:128, c:c + 1],
                            out_ps[:, 0:128, c:c + 1],
                            p0sb[:, j:j + 1, c:c + 1], None, ADD)
                    nc.scalar.copy(stvel[:, 0:128, :], out_ps[:, 128:256, :])
                else:
                    # pos carry plane p_in + Bv v_in on ACT (per-partition
                    # scale/bias); vel carry fused into the evacuation via
                    # DVE scalar_tensor_tensor: (Av * v_in) + psum.
                    psl = slice(128 * m - 1, 128 * m)
                    plp = planep.tile([128, 128, 2], f32, name=f"plp_{j}_{m}",
                                      tag="plp")
                    for c in range(2):
                        vin = stvel[:, psl, c:c + 1]
                        pin = stpos[:, psl, c:c + 1]
                        nc.scalar.activation(
                            plp[:, :, c:c + 1], bvrow_sb[:, :, :], IDENT,
                            bias=pin, scale=vin)
                        nc.vector.scalar_tensor_tensor(
                            stvel[:, csl, c:c + 1], avrow_sb[:, :, :],
                            vin, out_ps[:, 128:256, c:c + 1], MULT, ADD)
                    nc.vector.tensor_add(stpos[:, csl, :],
                                         out_ps[:, 0:128, :], plp[:, :, :])
            nc.sync.dma_start(pos_out[bsl, :], stpos[:, :, :])
            nc.gpsimd.dma_start(vel_out[bsl, :], stvel[:, :, :])

        # ---- software-pipelined main loop ----
        zf = {0: load_z(0), 1: load_z(1)}
        zb = {0: cast_z(0, zf[0])}
        transp(0, zb[0])
        for j in range(n_bt):
            if j + 2 < n_bt:
                zf[j + 2] = load_z(j + 2)
            if j + 1 < n_bt:
                zb[j + 1] = cast_z(j + 1, zf[j + 1])
                transp(j + 1, zb[j + 1])
            mains(j)

    nc.compile()
    return nc


# ----------------------------------------------------------------------------
# Host entry point
# ----------------------------------------------------------------------------

_CACHE = {}

# test-harness knobs (ignored in normal use)
PROFILE = False
LAST_RESULT = None


def _get_nc(n_bt):
    if n_bt not in _CACHE:
        _CACHE[n_bt] = build_nc(n_bt)
    return _CACHE[n_bt]


def kernel(pred_vel, dt, p0, q_pos, q_vel, r_vel):
    from concourse.bass_utils import run_bass_kernel_spmd

    z = np.ascontiguousarray(np.asarray(pred_vel, dtype=np.float32))
    p0 = np.ascontiguousarray(np.asarray(p0, dtype=np.float32))
    assert z.shape == (B_FULL, T, 2) and p0.shape == (B_FULL, 2)

    weights = build_weights(dt, q_pos, q_vel, r_vel)
    nc = _get_nc(B_CORE // 128)

    in_maps = []
    for i in range(N_CORES):
        sl = slice(i * B_CORE, (i + 1) * B_CORE)
        m = {"z_in": z[sl].reshape(B_CORE, 2 * T),
             "p0_in": p0[sl]}
        m.update(weights)
        in_maps.append(m)

    res = run_bass_kernel_spmd(nc, in_maps, core_ids=list(range(N_CORES)),
                               trace=PROFILE)
    global LAST_RESULT
    LAST_RESULT = res
    pos = np.concatenate([r["pos_out"].reshape(B_CORE, T, 2)
                          for r in res.results], axis=0)
    vel = np.concatenate([r["vel_out"].reshape(B_CORE, T, 2)
                          for r in res.results], axis=0)
    return pos, vel
